# revision 31
# baseline (speedup 1.0000x reference)
"""CrossAttention Trainium2 kernel.

Sharding: hybrid 2-way batch x 4-way heads over 8 cores. Core c owns batch
b=c//4 and head-group g=c%4 (4 of the 16 heads = 256 q/k/v channels, as two
head-pairs hp=0,1 of 128 channels each): q/k/v projections for its channels
over its batch's 2048 tokens, SDPA for its heads, and the out-projection
contribution of its 256 channels (row-sharded); the host sums 4 partials per
batch half. Versus pure head-TP this halves per-core input DMA (q/kv batch
slice) and output DMA (half the tokens), with identical PE/ACT totals.

Per-core device program (fp16 matmuls, fp32 PSUM accumulation):
  - qT/kvT arrive pre-transposed [hid, tok] so projection matmuls contract
    over the partition dim.
  - RMSNorm: squares on DVE (fp16 4x mode), chunk-tree-sum on Pool, one
    ones-vector matmul (partition reduce), sqrt on ACT, reciprocal on DVE.
    w_norm is folded into w_q on the host.
  - V is projected directly into natural [kv, ch] layout (input tile as the
    stationary operand, N=128 ch free). A ones column is appended so row 64
    of the P@V accumulator is the softmax denominator.
  - Scores are computed transposed ([kv, q]) so P^T feeds the P@V matmul
    directly; exp runs on ACT with the 1/sqrt(D) scale folded in. No
    max-subtraction: logits are O(6), well within fp16/fp32 exp range.
  - Window epilogue: unnormalized o is copied out of PSUM immediately (fast
    o_ps release); normalization happens in place on Pool off the critical
    path, before the out-projection consumes it.
  - out_proj contracts both head-pairs (K=256 via two accumulated matmuls)
    and emits outT [hid, tok] fp16 partials; host sums 4 partials per batch.
"""

from contextlib import ExitStack

import numpy as np
import ml_dtypes

import concourse.bacc as bacc
import concourse.bass as bass
import concourse.mybir as mybir
import concourse.tile as tile
from concourse import bass_utils

N_CORES = 8
B, SEQ, HID = 2, 2048, 1024
NH, D = 16, 64
GH = 4                   # head-groups (cores per batch shard)
CH = 256                 # q/k/v channels per core (4 heads, 2 head-pairs)
HC = HID // 128          # 8 hidden chunks of 128
PT = 512                 # projection tile (tokens)
NPT = SEQ // PT          # 4 token tiles per core
KT = SEQ // 128          # 16 kv tiles of 128
QW = 512                 # query window per scores tile
NQT = SEQ // QW          # 4 windows per core
VW = 72                  # vext inner stride (64 ch + ones col + pad)
EPS = 1e-5
F16 = mybir.dt.float16
F32 = mybir.dt.float32
AF = mybir.ActivationFunctionType


def emit_setup(tc, t_aps, ctx):
    """Pools + one-time weight loads/memsets, hoisted out of the loop."""
    nc = tc.nc
    st = {}
    st["singles"] = singles = ctx.enter_context(tc.tile_pool(name="singles", bufs=1))
    st["qin"] = ctx.enter_context(tc.tile_pool(name="qin", bufs=4))
    st["sqp"] = ctx.enter_context(tc.tile_pool(name="sqp", bufs=2))
    st["small"] = ctx.enter_context(tc.tile_pool(name="small", bufs=3))
    st["rstdp"] = ctx.enter_context(tc.tile_pool(name="rstdp", bufs=2))
    st["pTp"] = ctx.enter_context(tc.tile_pool(name="pTp", bufs=4))
    st["denp"] = ctx.enter_context(tc.tile_pool(name="denp", bufs=3))
    st["obp"] = ctx.enter_context(tc.tile_pool(name="obp", bufs=2))
    st["pp"] = ctx.enter_context(tc.tile_pool(name="pp", bufs=2, space="PSUM"))
    st["sp"] = ctx.enter_context(tc.tile_pool(name="sp", bufs=2, space="PSUM"))
    st["op"] = ctx.enter_context(tc.tile_pool(name="op", bufs=2, space="PSUM"))
    if True:
        # resident weights / activations
        wq_sb = singles.tile([128, HC, 2, 128], F16, tag="wq")
        wk_sb = singles.tile([128, HC, 2, 128], F16, tag="wk")
        wv_sb = singles.tile([128, HC, 2, 128], F16, tag="wv")
        wo_sb = singles.tile([128, 2, HC, 128], F16, tag="wo")
        bq_sb = singles.tile([128, 2], F32, tag="bq")
        bk_sb = singles.tile([128, 2], F32, tag="bk")
        bvrow = singles.tile([1, CH], F32, tag="bvrow")
        bvb_sb = singles.tile([128, 2, 2, D], F32, tag="bvb")
        ones_sb = singles.tile([128, 1], F16, tag="ones")
        eps_sb = singles.tile([1, 1], F32, tag="eps")
        kp_sb = singles.tile([128, 2, SEQ], F16, tag="kp")
        qp_sb = singles.tile([128, 2, SEQ], F16, tag="qp")
        # natural-layout V (+ ones col) per (kv-tile, head-pair, head)
        vext_sb = singles.tile([128, KT, 2, 2, VW], F16, tag="vext")
        o_sb = singles.tile([128, 2, SEQ], F16, tag="osb")

        # weights go out on the ACT-issued DMA queue so the serial SP queue
        # starts with the first input tiles (shorter compute lead-in); wk
        # first since k-projection is the earliest weight consumer
        nc.scalar.dma_start(wk_sb[:], t_aps["wkT"])
        nc.scalar.dma_start(wq_sb[:], t_aps["wqT"])
        nc.scalar.dma_start(wv_sb[:], t_aps["wvT"])
        nc.scalar.dma_start(wo_sb[:], t_aps["woT"])
        nc.scalar.dma_start(bq_sb[:], t_aps["bq"])
        nc.scalar.dma_start(bk_sb[:], t_aps["bk"])
        nc.scalar.dma_start(bvrow[:], t_aps["bv"])
        nc.gpsimd.partition_broadcast(bvb_sb[:], bvrow[:])
        nc.vector.memset(ones_sb[:], 1.0)
        nc.vector.memset(eps_sb[:], EPS)
        nc.vector.memset(vext_sb[:, :, :, :, D : D + 1], 1.0)
    for k in ("wq_sb", "wk_sb", "wv_sb", "wo_sb", "bq_sb", "bk_sb",
              "bvb_sb", "ones_sb", "eps_sb", "kp_sb", "qp_sb", "vext_sb",
              "o_sb"):
        st[k] = locals()[k]
    return st


def emit_body(tc, t_aps, st, parts="abc"):
    nc = tc.nc
    qT = t_aps["qT"]
    kvT = t_aps["kvT"]
    outT = t_aps["outT"]
    qin = st["qin"]; sqp = st["sqp"]; small = st["small"]
    rstdp = st["rstdp"]; pTp = st["pTp"]; denp = st["denp"]
    obp = st["obp"]; pp = st["pp"]; sp = st["sp"]; op = st["op"]
    wq_sb = st["wq_sb"]; wk_sb = st["wk_sb"]; wv_sb = st["wv_sb"]
    wo_sb = st["wo_sb"]; bq_sb = st["bq_sb"]; bk_sb = st["bk_sb"]
    bvb_sb = st["bvb_sb"]; ones_sb = st["ones_sb"]; eps_sb = st["eps_sb"]
    kp_sb = st["kp_sb"]; qp_sb = st["qp_sb"]; vext_sb = st["vext_sb"]
    o_sb = st["o_sb"]
    if True:
        # ---- Phase A: projections + RMSNorm stats, tiled over tokens ----
        def phase_a(t):
            ts = t * PT
            qt_t = qin.tile([128, HC, PT], F16, tag="qt")
            kvt_t = qin.tile([128, HC, PT], F16, tag="kvt")
            nc.sync.dma_start(qt_t[:], qT[:, :, ts : ts + PT])
            nc.sync.dma_start(kvt_t[:], kvT[:, :, ts : ts + PT])

            # sum of squares over hidden: squares on DVE (fp16 4x), chunk
            # tree-sum on Pool, then one ones-matmul (partition reduce)
            sq_t = sqp.tile([128, HC, PT], F16, tag="sq")
            nc.vector.tensor_mul(sq_t[:], qt_t[:], qt_t[:])
            for lvl in (1, 2, 4):
                for c in range(0, HC, 2 * lvl):
                    nc.vector.tensor_add(
                        sq_t[:, c, :], sq_t[:, c, :], sq_t[:, c + lvl, :]
                    )
            ms_ps = pp.tile([1, PT], F32, tag="pp")
            nc.tensor.matmul(
                ms_ps[:], ones_sb[:], sq_t[:, 0, :], start=True, stop=True
            )
            # rstd = 1/sqrt(ms/HID + eps): sqrt on ACT (reads PSUM, folds
            # scale+eps), reciprocal on DVE
            sx = small.tile([1, PT], F32, tag="sx")
            nc.scalar.activation(
                sx[:], ms_ps[:], AF.Sqrt, bias=eps_sb[:], scale=1.0 / HID
            )
            y = small.tile([1, PT], F32, tag="y")
            nc.vector.reciprocal(y[:], sx[:])
            rstd_b = rstdp.tile([128, PT], F32, tag="rstd_b")
            nc.gpsimd.partition_broadcast(rstd_b[:], y[:])

            for hp in range(2):
                # k-projection -> K^T [ch, tok]
                kp_ps = pp.tile([128, PT], F32, tag="pp")
                for c in range(HC):
                    nc.tensor.matmul(
                        kp_ps[:], wk_sb[:, c, hp, :], kvt_t[:, c, :],
                        start=(c == 0), stop=(c == HC - 1),
                    )
                nc.vector.tensor_scalar_add(
                    kp_sb[:, hp, ts : ts + PT], kp_ps[:],
                    bk_sb[:, hp : hp + 1],
                )

                # q-projection -> Q^T [ch, tok], scaled by rstd then + b_q
                qp_ps = pp.tile([128, PT], F32, tag="pp")
                for c in range(HC):
                    nc.tensor.matmul(
                        qp_ps[:], wq_sb[:, c, hp, :], qt_t[:, c, :],
                        start=(c == 0), stop=(c == HC - 1),
                    )
                nc.vector.tensor_mul(
                    qp_sb[:, hp, ts : ts + PT], qp_ps[:], rstd_b[:]
                )
                nc.vector.tensor_scalar_add(
                    qp_sb[:, hp, ts : ts + PT], qp_sb[:, hp, ts : ts + PT],
                    bq_sb[:, hp : hp + 1],
                )

            # v-projection directly into natural [kv, ch] layout: the input
            # tile is the stationary operand, w_v streams (N=128 ch)
            for i in range(PT // 128):
                g = t * (PT // 128) + i
                for hp in range(2):
                    vp_ps = pp.tile([128, 2, D], F32, tag="pp",
                                    name=f"vp{i}_{hp}")
                    for c in range(HC):
                        nc.tensor.matmul(
                            vp_ps[:],
                            kvt_t[:, c, i * 128 : (i + 1) * 128],
                            wv_sb[:, c, hp, :],
                            start=(c == 0), stop=(c == HC - 1),
                        )
                    nc.vector.tensor_add(
                        vext_sb[:, g, hp, :, 0:D], vp_ps[:], bvb_sb[:, hp]
                    )

        # ---- Phase B: attention per (head-pair, q-window) ----
        o_ps_live = {}

        def phase_b_chunk(hp, qt, kt_lo, kt_hi, last=False):
            qs = qt * QW
            if kt_lo == 0:
                o_ps_live[(hp, qt)] = [
                    op.tile([D + 1, QW], F32, tag="op",
                            name=f"o_ps{hp}_{qt}_{h}")
                    for h in range(2)
                ]
            o_ps = o_ps_live[(hp, qt)]
            for kt in range(kt_lo, kt_hi):
                kv0 = kt * 128
                s_ps = sp.tile([128, 2, QW], F32, tag="sp")
                for h in range(2):
                    nc.tensor.matmul(
                        s_ps[:, h, :],
                        kp_sb[h * D : (h + 1) * D, hp, kv0 : kv0 + 128],
                        qp_sb[h * D : (h + 1) * D, hp, qs : qs + QW],
                        start=True, stop=True,
                    )
                pT = pTp.tile([128, 2, QW], F16, tag="pT")
                nc.scalar.activation(pT[:], s_ps[:], AF.Exp, scale=D ** -0.5)
                for h in range(2):
                    nc.tensor.matmul(
                        o_ps[h][:],
                        vext_sb[:, kt, hp, h, 0 : D + 1],
                        pT[:, h, :],
                        start=(kt == 0), stop=(kt == KT - 1),
                    )
            if kt_hi == KT:
                # early PSUM release: copy unnormalized o out immediately,
                # then normalize o_sb in place (Pool) off the critical path;
                # o_ps frees after copy+recip only.
                for h in range(2):
                    recip = small.tile([1, QW], F32, tag="recip")
                    nc.vector.reciprocal(recip[:], o_ps[h][D : D + 1, :])
                    (nc.scalar.copy if last else nc.vector.tensor_copy)(
                        o_sb[h * D : (h + 1) * D, hp, qs : qs + QW],
                        o_ps[h][0:D, :],
                    )
                    # full-partition broadcast; the per-head mul slices both
                    # operands at the same base partition (IBIR297)
                    den = denp.tile([128, QW], F32, tag="den")
                    nc.gpsimd.partition_broadcast(den[:], recip[:])
                    nc.vector.tensor_mul(
                        o_sb[h * D : (h + 1) * D, hp, qs : qs + QW],
                        o_sb[h * D : (h + 1) * D, hp, qs : qs + QW],
                        den[h * D : (h + 1) * D, :],
                    )
                del o_ps_live[(hp, qt)]

        def phase_b(hp, qt):
            phase_b_chunk(hp, qt, 0, KT)

        # out-projection for a q-window: contract both head-pairs (K=256);
        # all 8 out-chunks staged into one SBUF tile -> a single DMA
        def phase_c(qt, ms=None):
            qs = qt * QW
            ms = list(range(HC)) if ms is None else ms
            ob = obp.tile([128, len(ms), QW], F16, tag="ob")
            # the very last window runs after all exp work: alternate its
            # PSUM->SBUF copies between the then-idle ACT engine and DVE
            last = qt == NQT - 1
            for mi, m in enumerate(ms):
                out_ps = pp.tile([128, QW], F32, tag="pp")
                for hp in range(2):
                    nc.tensor.matmul(
                        out_ps[:], wo_sb[:, hp, m, :],
                        o_sb[:, hp, qs : qs + QW],
                        start=(hp == 0), stop=(hp == 1),
                    )
                if last and m % 2 == 0:
                    nc.scalar.copy(ob[:, mi, :], out_ps[:])
                else:
                    nc.vector.tensor_copy(ob[:, mi, :], out_ps[:])
            # output stores issue from the gpsimd queue so they never block
            # the next iteration's input loads on the in-order SP queue
            nc.gpsimd.dma_start(
                outT[:, ms[0] : ms[0] + len(ms), qs : qs + QW], ob[:]
            )

        # ablation stubs: fill tensors a disabled phase would produce
        if "a" not in parts:
            nc.vector.memset(kp_sb[:], 0.01)
            nc.vector.memset(qp_sb[:], 0.01)
            nc.vector.memset(vext_sb[:], 0.01)
        if "b" not in parts:
            nc.vector.memset(o_sb[:], 0.01)

        if parts == "abc":
            # interleave: kt-granular head so window (hp0,0) starts after 2
            # projection tiles; phase_c woven between windows; tail weaves
            # out-proj of window 2 into window (hp1,3)'s kt stream
            phase_a(0)
            phase_a(1)
            phase_b_chunk(0, 0, 0, 8)
            phase_a(2)
            phase_b_chunk(0, 0, 8, 12)
            phase_a(3)
            phase_b_chunk(0, 0, 12, KT)
            phase_b(1, 0)
            phase_c(0)
            phase_b(0, 1)
            phase_b(1, 1)
            phase_c(1)
            phase_b(0, 2)
            phase_b(1, 2)
            # weave the tail: out-proj of window 2 inside window 3's kt
            # stream so only one window's out-proj remains after the last exp
            phase_b_chunk(0, 3, 0, 6)
            phase_c(2, ms=[0, 1, 2, 3])
            phase_b_chunk(0, 3, 6, 11)
            phase_c(2, ms=[4, 5, 6, 7])
            phase_b_chunk(0, 3, 11, KT)
            phase_b_chunk(1, 3, 0, 10)
            phase_b_chunk(1, 3, 10, KT, last=True)
            phase_c(3)
        else:
            if "a" in parts:
                for t in range(NPT):
                    phase_a(t)
            for hp in range(2):
                for qt in range(NQT):
                    if "b" in parts:
                        phase_b(hp, qt)
            for qt in range(NQT):
                if "c" in parts:
                    phase_c(qt)
            if "c" not in parts:
                for qt in range(NQT):
                    qs = qt * QW
                    ob = obp.tile([128, HC, QW], F16, tag="ob")
                    nc.vector.memset(ob[:], 0.0)
                    nc.gpsimd.dma_start(outT[:, :, qs : qs + QW], ob[:])


def build_program(loop_n=None, parts="abc", body_reps=1):
    nc = bacc.Bacc("TRN2", target_bir_lowering=False, debug=False,
                   num_devices=N_CORES)
    specs = [
        ("qT", (128, HC, SEQ), F16, "ExternalInput"),
        ("kvT", (128, HC, SEQ), F16, "ExternalInput"),
        ("wqT", (128, HC, 2, 128), F16, "ExternalInput"),
        ("wkT", (128, HC, 2, 128), F16, "ExternalInput"),
        ("wvT", (128, HC, 2, 128), F16, "ExternalInput"),
        ("woT", (128, 2, HC, 128), F16, "ExternalInput"),
        ("bq", (128, 2), F32, "ExternalInput"),
        ("bk", (128, 2), F32, "ExternalInput"),
        ("bv", (1, CH), F32, "ExternalInput"),
        ("outT", (128, HC, SEQ), F16, "ExternalOutput"),
    ]
    t_aps = {}
    for name, shape, dt_, kind in specs:
        t_aps[name] = nc.dram_tensor(name, shape, dt_, kind=kind).ap()
    with tile.TileContext(nc) as tc:
        if loop_n is not None:
            # timing variant: self-contained body inside the hardware loop
            # (pools/weights in-body; loop-invariant tiles interact badly
            # with For_i)
            hints = (
                mybir.EngineType.PE, mybir.EngineType.DVE,
                mybir.EngineType.Activation, mybir.EngineType.Pool,
                mybir.EngineType.SP,
            )
            with tc.For_i(0, loop_n, 1, hint_engines=hints):
                with ExitStack() as ctx:
                    st = emit_setup(tc, t_aps, ctx)
                    for _ in range(body_reps):
                        emit_body(tc, t_aps, st, parts=parts)
        else:
            with ExitStack() as ctx:
                st = emit_setup(tc, t_aps, ctx)
                for _ in range(body_reps):
                    emit_body(tc, t_aps, st, parts=parts)
    nc.compile()
    return nc


def prep_inputs(q, kv, w_norm, w_q, b_q, w_kv, b_kv, w_out, b_out):
    """Host-side shard prep: transpose/cast/slice the full inputs per core."""
    f16 = ml_dtypes.float16 if hasattr(ml_dtypes, "float16") else np.float16

    def to_chunked_T(x2d):
        # [tok, hid] -> [128, hid//128, tok]
        tok, hid = x2d.shape
        return np.ascontiguousarray(
            x2d.T.reshape(hid // 128, 128, tok).transpose(1, 0, 2)
        )

    q = np.asarray(q, np.float32)
    kv = np.asarray(kv, np.float32)
    w_norm = np.asarray(w_norm, np.float32)
    w_q = np.asarray(w_q, np.float32)
    b_q = np.asarray(b_q, np.float32)
    w_kv = np.asarray(w_kv, np.float32)
    b_kv = np.asarray(b_kv, np.float32)
    w_out = np.asarray(w_out, np.float32)

    qT_b = [to_chunked_T(q[b]).astype(f16) for b in range(B)]
    kvT_b = [to_chunked_T(kv[b]).astype(f16) for b in range(B)]
    w_q_eff = w_q * w_norm[None, :]

    in_maps = []
    for core in range(N_CORES):
        b, g = core // GH, core % GH
        r0 = CH * g
        # [out_ch(256), in_hid] slices -> [128, HC, 256] -> [128, HC, 2, 128]
        wq_c = to_chunked_T(w_q_eff[r0 : r0 + CH]).reshape(128, HC, 2, 128)
        wk_c = to_chunked_T(w_kv[r0 : r0 + CH]).reshape(128, HC, 2, 128)
        wv_c = to_chunked_T(
            w_kv[HID + r0 : HID + r0 + CH]
        ).reshape(128, HC, 2, 128)
        # wo_sb[p, hp, m, j] = w_out[m*128+j, r0 + hp*128 + p]
        wo_c = np.ascontiguousarray(
            w_out[:, r0 : r0 + CH].T.reshape(2, 128, HC, 128).transpose(1, 0, 2, 3)
        )
        in_maps.append({
            "qT": qT_b[b],
            "kvT": kvT_b[b],
            "wqT": np.ascontiguousarray(wq_c).astype(f16),
            "wkT": np.ascontiguousarray(wk_c).astype(f16),
            "wvT": np.ascontiguousarray(wv_c).astype(f16),
            "woT": wo_c.astype(f16),
            "bq": np.ascontiguousarray(
                b_q[r0 : r0 + CH].reshape(2, 128).T
            ).copy(),
            "bk": np.ascontiguousarray(
                b_kv[r0 : r0 + CH].reshape(2, 128).T
            ).copy(),
            "bv": b_kv[HID + r0 : HID + r0 + CH].reshape(1, CH).copy(),
        })
    return in_maps


_CACHE = {}


def _get_nc():
    if "nc" not in _CACHE:
        _CACHE["nc"] = build_program()
    return _CACHE["nc"]


def gather_output(results, b_out):
    out = np.zeros((B, HID, SEQ), np.float64)
    for core in range(len(results)):
        b = core // GH
        # outT [128, HC, SEQ]: out-channel = m*128 + p
        part = results[core]["outT"].astype(np.float64)
        out[b] += part.transpose(1, 0, 2).reshape(HID, SEQ)
    out = out.transpose(0, 2, 1).astype(np.float32)
    out += np.asarray(b_out, np.float32)[None, None, :]
    return np.ascontiguousarray(out)


def kernel(q, kv, w_norm, w_q, b_q, w_kv, b_kv, w_out, b_out):
    nc = _get_nc()
    in_maps = prep_inputs(q, kv, w_norm, w_q, b_q, w_kv, b_kv, w_out, b_out)
    res = bass_utils.run_bass_kernel_spmd(nc, in_maps, core_ids=list(range(N_CORES)))
    return gather_output(res.results, b_out)
